# revision 3
# baseline (speedup 1.0000x reference)
"""GCN (2x GCNConv + FC) on Trainium2, 8-core SPMD Bass kernel.

Math (per layer): out = D^{-1/2} (A + I) D^{-1/2} (x @ W) + b, D = indeg + 1.
The two D^{-1/2} are folded into a host pre-scale of x rows and a device
post-scale of the aggregation (positive scales commute with relu).

Sharding: nodes split 8 ways by dst (6250/core). Per dst-block of 128 nodes,
edges are processed in 128-edge chunks: a one-hot selection matrix S (built
on the vector engine via is_equal against an iota row) scatters gathered
source rows into PSUM on the tensor engine. Source rows are fetched with
SWDGE dma_gather on 4 queues. Layer 1 aggregates raw pre-scaled x and applies
W1 after aggregation (linearity), so no y1 materialization is needed.
Hidden states for layer 2 are exchanged with an AllGather collective.
"""
import numpy as np
import ml_dtypes

N_CORES = 8
N = 50000
FEAT = 128
HID = 64
NCLS = 12
PC = N // N_CORES          # 6250 nodes per core
NBLK = (PC + 127) // 128   # 49 dst blocks per core
PCP = NBLK * 128           # 6272 padded rows
HALF = 25000               # int16 gather-index split point
CHUNK = 128
BATCH = 1024               # edges per dma_gather (HW cap at elem_size=128)
BPC = BATCH // CHUNK       # chunks per gather batch = 8
PAD_OFF = 200.0            # dst offset that matches no one-hot column

bf16 = ml_dtypes.bfloat16


def _prep(x, edge_index, W1, b1, W2, b2, Wfc, bfc):
    """Host-side preprocessing: degrees, edge partitioning, layouts."""
    src = np.asarray(edge_index[0], dtype=np.int64)
    dst = np.asarray(edge_index[1], dtype=np.int64)

    deg = np.bincount(dst, minlength=N).astype(np.float64) + 1.0
    dinv = (1.0 / np.sqrt(deg)).astype(np.float32)

    x_s = (np.asarray(x, np.float32) * dinv[:, None]).astype(bf16)  # [N,128]

    core = dst // PC
    local = dst - core * PC
    blk = local // 128
    off = (local % 128).astype(np.float32)
    half = (src >= HALF).astype(np.int64)

    key = (core * NBLK + blk) * 2 + half
    order = np.argsort(key, kind="stable")
    cnt = np.bincount(key, minlength=N_CORES * NBLK * 2).reshape(
        N_CORES, NBLK, 2)
    # uniform chunk counts across cores (one SPMD program)
    CC = np.maximum(1, (cnt.max(axis=0) + CHUNK - 1) // CHUNK)  # [NBLK, 2]
    nch = (int(CC[:, 0].sum()), int(CC[:, 1].sum()))

    gstart = np.zeros(N_CORES * NBLK * 2 + 1, np.int64)
    np.cumsum(np.bincount(key, minlength=N_CORES * NBLK * 2), out=gstart[1:])
    src_sorted = src[order]
    off_sorted = off[order]

    in_maps = []
    for c in range(N_CORES):
        idx_streams = {}
        dst_streams = {}
        for h in (0, 1):
            nslots = nch[h] * CHUNK
            idx_arr = np.zeros(nslots, np.int16)
            off_arr = np.full(nslots, PAD_OFF, np.float32)
            pos = 0
            for b in range(NBLK):
                k = (c * NBLK + b) * 2 + h
                g0, g1 = gstart[k], gstart[k + 1]
                n = int(g1 - g0)
                idx_arr[pos:pos + n] = (src_sorted[g0:g1] - h * HALF).astype(
                    np.int16)
                off_arr[pos:pos + n] = off_sorted[g0:g1]
                pos += int(CC[b, h]) * CHUNK
            nb = (nch[h] + BPC - 1) // BPC
            idx_pad = np.zeros(nb * BATCH, np.int16)
            idx_pad[:nslots] = idx_arr
            # wrapped layout per 1024-batch: idx j -> partition j%16, col j//16
            w = idx_pad.reshape(nb, BATCH // 16, 16).transpose(0, 2, 1)
            idx_tile = np.tile(w, (1, 8, 1)).reshape(nb, 128, BATCH // 16)
            idx_tile = idx_tile.transpose(1, 0, 2).reshape(
                128, nb * BATCH // 16)
            idx_streams[h] = np.ascontiguousarray(idx_tile)
            dst_streams[h] = np.ascontiguousarray(
                off_arr.reshape(nch[h], CHUNK).T)  # [128, nch]

        dl = dinv[c * PC:(c + 1) * PC]
        dinv_pad = np.zeros(PCP, np.float32)
        dinv_pad[:PC] = dl
        sq_pad = np.zeros(PCP, np.float32)
        sq_pad[:PC] = 1.0 / dl
        x_own = np.zeros((PCP, FEAT), bf16)
        x_own[:PC] = x_s[c * PC:(c + 1) * PC]

        im = {
            "x_s": x_s,
            "x_own": x_own,
            "idxA": idx_streams[0], "idxB": idx_streams[1],
            "dstA": dst_streams[0], "dstB": dst_streams[1],
            "W1": np.asarray(W1, np.float32).astype(bf16),
            "W2": np.asarray(W2, np.float32).astype(bf16),
            "Wfc": np.asarray(Wfc, np.float32).astype(bf16),
            "b1": np.asarray(b1, np.float32).astype(bf16)[None, :],
            "b2": np.asarray(b2, np.float32).astype(bf16)[None, :],
            "bfc": np.asarray(bfc, np.float32).astype(bf16)[None, :],
            "sqdeg": sq_pad.astype(bf16)[None, :],
            "dinv2T": np.ascontiguousarray(
                (dinv_pad ** 2).reshape(NBLK, 128).T.astype(np.float32)),
            "dinvT": np.ascontiguousarray(
                dinv_pad.reshape(NBLK, 128).T.astype(np.float32)),
            "iota": np.tile(np.arange(128, dtype=bf16)[None, :], (128, 1)),
            "ident": np.eye(128, dtype=bf16),
            "ones": np.ones((1, 128), bf16),
        }
        in_maps.append(im)

    meta = {"CC": CC, "nchA": nch[0], "nchB": nch[1]}
    return in_maps, meta


def _build(meta):
    import concourse.bacc as bacc
    import concourse.tile as tile
    from concourse import mybir

    CC = meta["CC"]
    nchA, nchB = meta["nchA"], meta["nchB"]
    nbA = (nchA + BPC - 1) // BPC
    nbB = (nchB + BPC - 1) // BPC

    nc = bacc.Bacc("TRN2", target_bir_lowering=False, debug=False,
                   num_devices=N_CORES, num_swdge_queues=4)
    f32, i16, bft = mybir.dt.float32, mybir.dt.int16, mybir.dt.bfloat16
    AO = mybir.AluOpType

    x_s = nc.dram_tensor("x_s", [N, FEAT], bft, kind="ExternalInput")
    x_own = nc.dram_tensor("x_own", [PCP, FEAT], bft, kind="ExternalInput")
    idxA = nc.dram_tensor("idxA", [128, nbA * BATCH // 16], i16,
                          kind="ExternalInput")
    idxB = nc.dram_tensor("idxB", [128, nbB * BATCH // 16], i16,
                          kind="ExternalInput")
    dstA = nc.dram_tensor("dstA", [128, nchA], f32, kind="ExternalInput")
    dstB = nc.dram_tensor("dstB", [128, nchB], f32, kind="ExternalInput")
    W1 = nc.dram_tensor("W1", [FEAT, HID], bft, kind="ExternalInput")
    W2 = nc.dram_tensor("W2", [HID, HID], bft, kind="ExternalInput")
    Wfc = nc.dram_tensor("Wfc", [HID, NCLS], bft, kind="ExternalInput")
    b1 = nc.dram_tensor("b1", [1, HID], bft, kind="ExternalInput")
    b2 = nc.dram_tensor("b2", [1, HID], bft, kind="ExternalInput")
    bfc = nc.dram_tensor("bfc", [1, NCLS], bft, kind="ExternalInput")
    sqdeg = nc.dram_tensor("sqdeg", [1, PCP], bft, kind="ExternalInput")
    dinv2T = nc.dram_tensor("dinv2T", [128, NBLK], f32, kind="ExternalInput")
    dinvT = nc.dram_tensor("dinvT", [128, NBLK], f32, kind="ExternalInput")
    iota = nc.dram_tensor("iota", [128, 128], bft, kind="ExternalInput")
    ident = nc.dram_tensor("ident", [128, 128], bft, kind="ExternalInput")
    ones = nc.dram_tensor("ones", [1, 128], bft, kind="ExternalInput")

    out = nc.dram_tensor("out", [PCP, NCLS], f32, kind="ExternalOutput")

    y2_local = nc.dram_tensor("y2_local", [PC, 128], bft, kind="Internal")
    y2_full = nc.dram_tensor("y2_full", [N, 128], bft, kind="Internal",
                             addr_space="Shared")

    with tile.TileContext(nc) as tc:
        cp = tc.alloc_tile_pool(name="const", bufs=1)
        y2k = tc.alloc_tile_pool(name="y2keep", bufs=1)

        def load_const(name, dram, shape, dt):
            t = cp.tile(shape, dt, tag=name)
            nc.sync.dma_start(out=t[:], in_=dram[:, :])
            return t

        iota_t = load_const("iota", iota, [128, 128], bft)
        ident_t = load_const("ident", ident, [128, 128], bft)
        ones_t = load_const("ones", ones, [1, 128], bft)
        W1_t = load_const("W1", W1, [FEAT, HID], bft)
        W2_t = load_const("W2", W2, [HID, HID], bft)
        Wfc_t = load_const("Wfc", Wfc, [HID, NCLS], bft)
        b1_t = load_const("b1", b1, [1, HID], bft)
        b2_t = load_const("b2", b2, [1, HID], bft)
        bfc_t = load_const("bfc", bfc, [1, NCLS], bft)
        sq_t = load_const("sqdeg", sqdeg, [1, PCP], bft)
        d2_t = load_const("dinv2T", dinv2T, [128, NBLK], f32)
        d1_t = load_const("dinvT", dinvT, [128, NBLK], f32)
        idxA_t = load_const("idxA", idxA, [128, nbA * BATCH // 16], i16)
        idxB_t = load_const("idxB", idxB, [128, nbB * BATCH // 16], i16)
        dstA_t = load_const("dstA", dstA, [128, nchA], f32)
        dstB_t = load_const("dstB", dstB, [128, nchB], f32)

        gp = tc.alloc_tile_pool(name="g", bufs=8)
        sp = tc.alloc_tile_pool(name="s", bufs=6)
        xop = tc.alloc_tile_pool(name="xown", bufs=2)
        zxp = tc.alloc_tile_pool(name="zx", bufs=2, space="PSUM")
        z1p = tc.alloc_tile_pool(name="z1", bufs=2, space="PSUM")
        trp_ = tc.alloc_tile_pool(name="tr", bufs=1, space="PSUM")
        y2psp = tc.alloc_tile_pool(name="y2ps", bufs=1, space="PSUM")
        zxsbp = tc.alloc_tile_pool(name="zxsb", bufs=2)
        y2pp = tc.alloc_tile_pool(name="y2p", bufs=2)
        y2pTp = tc.alloc_tile_pool(name="y2pT", bufs=2)
        osbp = tc.alloc_tile_pool(name="osb", bufs=2)

        y2_tiles = []
        qctr = [0]

        def emit_layer(layer, gsrcA, gsrcB):
            batches = {0: {}, 1: {}}

            def get_batch(hlf, bi):
                d = batches[hlf]
                if bi in d:
                    return d[bi]
                g_t = gp.tile([128, BPC, FEAT], bft, tag="g")
                it = idxA_t if hlf == 0 else idxB_t
                srcap = gsrcA if hlf == 0 else gsrcB
                nc.gpsimd.dma_gather(
                    out_ap=g_t[:],
                    in_ap=srcap,
                    idxs_ap=it[:, bi * (BATCH // 16):(bi + 1) * (BATCH // 16)],
                    num_idxs=BATCH, num_idxs_reg=BATCH, elem_size=FEAT,
                    queue_num=qctr[0] % 4)
                qctr[0] += 1
                d[bi] = g_t
                for old in [k for k in d if k < bi - 2]:
                    del d[old]
                return g_t

            chunk_base = [0, 0]
            for b in range(NBLK):
                zx = zxp.tile([128, 128 if layer == 1 else HID], f32,
                              space="PSUM", tag="zx")
                first = True
                for hlf in (0, 1):
                    nch_blk = int(CC[b, hlf])
                    dst_t = dstA_t if hlf == 0 else dstB_t
                    for k in range(nch_blk):
                        ci = chunk_base[hlf] + k
                        g_t = get_batch(hlf, ci // BPC)
                        cw = ci % BPC
                        s_t = sp.tile([128, 128], bft, tag="s")
                        nc.vector.tensor_scalar(
                            s_t[:], iota_t[:], dst_t[:, ci:ci + 1], None,
                            AO.is_equal)
                        if layer == 1:
                            # ZxT[feat, dst] += G^T @ S
                            nc.tensor.matmul(
                                out=zx[:], lhsT=g_t[:, cw, :], rhs=s_t[:],
                                start=first, stop=False)
                        else:
                            # Z2[dst, hid] += S^T @ G[:, :HID]
                            nc.tensor.matmul(
                                out=zx[:], lhsT=s_t[:],
                                rhs=g_t[:, cw, 0:HID],
                                start=first, stop=False)
                        first = False
                    chunk_base[hlf] += nch_blk

                # self-loop term closes the accumulation group
                if layer == 1:
                    xo = xop.tile([128, FEAT], bft, tag="xown")
                    nc.sync.dma_start(
                        out=xo[:], in_=x_own[b * 128:(b + 1) * 128, :])
                    nc.tensor.matmul(out=zx[:], lhsT=xo[:], rhs=ident_t[:],
                                     start=first, stop=True)
                else:
                    nc.tensor.matmul(out=zx[:], lhsT=ident_t[:],
                                     rhs=y2_tiles[b][:, 0:HID],
                                     start=first, stop=True)

                if layer == 1:
                    zxs = zxsbp.tile([128, 128], bft, tag="zxsb")
                    if b % 2 == 0:
                        nc.vector.tensor_copy(out=zxs[:], in_=zx[:])
                    else:
                        nc.any.tensor_copy(out=zxs[:], in_=zx[:])
                    z1 = z1p.tile([128, HID], f32, space="PSUM", tag="z1")
                    nc.tensor.matmul(out=z1[:], lhsT=zxs[:], rhs=W1_t[:],
                                     start=True, stop=False)
                    nc.tensor.matmul(
                        out=z1[:], lhsT=sq_t[0:1, b * 128:(b + 1) * 128],
                        rhs=b1_t[:], start=False, stop=True)
                    y2p = y2pp.tile([128, HID], bft, tag="y2p")
                    nc.vector.tensor_scalar(
                        y2p[:], z1[:], 0.0, d2_t[:, b:b + 1], AO.max, AO.mult)
                    trp = trp_.tile([HID, 128], bft, space="PSUM", tag="tr")
                    nc.tensor.transpose(out=trp[:], in_=y2p[:],
                                        identity=ident_t[:])
                    y2pT = y2pTp.tile([HID, 128], bft, tag="y2pT")
                    nc.any.tensor_copy(out=y2pT[:], in_=trp[:])
                    y2ps = y2psp.tile([128, HID], f32, space="PSUM",
                                      tag="y2ps")
                    nc.tensor.matmul(out=y2ps[:], lhsT=y2pT[:], rhs=W2_t[:],
                                     start=True, stop=True)
                    y2s = y2k.tile([128, 128], bft, tag=f"y2_{b}")
                    nc.vector.tensor_copy(out=y2s[:, 0:HID], in_=y2ps[:])
                    y2_tiles.append(y2s)
                    r0 = b * 128
                    rows = min(128, PC - r0)
                    nc.sync.dma_start(out=y2_local[r0:r0 + rows, :],
                                      in_=y2s[0:rows, :])
                else:
                    h2 = y2pp.tile([128, HID], bft, tag="h2")
                    nc.vector.tensor_scalar(
                        h2[:], zx[:], 0.0, d1_t[:, b:b + 1], AO.max, AO.mult)
                    trp = trp_.tile([HID, 128], bft, space="PSUM", tag="tr")
                    nc.tensor.transpose(out=trp[:], in_=h2[:],
                                        identity=ident_t[:])
                    h2T = y2pTp.tile([HID, 128], bft, tag="h2T")
                    nc.any.tensor_copy(out=h2T[:], in_=trp[:])
                    op = z1p.tile([128, NCLS], f32, space="PSUM", tag="op")
                    nc.tensor.matmul(out=op[:], lhsT=h2T[:], rhs=Wfc_t[:],
                                     start=True, stop=False)
                    nc.tensor.matmul(out=op[:], lhsT=ones_t[:], rhs=bfc_t[:],
                                     start=False, stop=True)
                    osb = osbp.tile([128, NCLS], f32, tag="osb")
                    nc.vector.tensor_copy(out=osb[:], in_=op[:])
                    nc.sync.dma_start(out=out[b * 128:(b + 1) * 128, :],
                                      in_=osb[:])

        emit_layer(1, x_s[0:HALF, :], x_s[HALF:N, :])
        nc.gpsimd.collective_compute(
            "AllGather", AO.bypass,
            replica_groups=[list(range(N_CORES))],
            ins=[y2_local[:, :]], outs=[y2_full[:, :]])
        emit_layer(2, y2_full[0:HALF, :], y2_full[HALF:N, :])

        for p in (osbp, y2pTp, y2pp, zxsbp, y2psp, trp_, z1p, zxp, xop, sp,
                  gp, y2k, cp):
            p.release()

    nc.compile()
    return nc


def kernel(**inputs):
    from concourse import bass_utils

    in_maps, meta = _prep(**inputs)
    nc = _build(meta)
    res = bass_utils.run_bass_kernel_spmd(
        nc, in_maps, core_ids=list(range(N_CORES)))
    out = np.concatenate(
        [np.asarray(res.results[c]["out"])[:PC] for c in range(N_CORES)],
        axis=0)
    return out.astype(np.float32)


# revision 4
# speedup vs baseline: 1.3288x; 1.3288x over previous
"""GCN (2x GCNConv + FC) on Trainium2, 8-core SPMD Bass kernel.

Math (per layer): out = D^{-1/2} (A + I) D^{-1/2} (x @ W) + b, D = indeg + 1.
The two D^{-1/2} are folded into a host pre-scale of x rows and a device
post-scale of the aggregation (positive scales commute with relu).

Sharding: nodes split 8 ways by dst (6250/core). Per dst-block of 128 nodes,
edges are processed in 128-edge chunks: a one-hot selection matrix S (built
on the vector engine via is_equal against an iota row) scatters gathered
source rows into PSUM on the tensor engine. Source rows are fetched with
SWDGE dma_gather on 4 queues. Layer 1 aggregates raw pre-scaled x and applies
W1 after aggregation (linearity), so no y1 materialization is needed.
Hidden states for layer 2 are exchanged with an AllGather collective.
"""
import numpy as np
import ml_dtypes

N_CORES = 8
N = 50000
FEAT = 128
HID = 64
NCLS = 12
PC = N // N_CORES          # 6250 nodes per core
NBLK = (PC + 127) // 128   # 49 dst blocks per core
PCP = NBLK * 128           # 6272 padded rows
HALF = 25000               # int16 gather-index split point
CHUNK = 128
BATCH = 1024               # edges per dma_gather (HW cap at elem_size=128)
BPC = BATCH // CHUNK       # chunks per gather batch = 8
PAD_OFF = 200.0            # dst offset that matches no one-hot column

bf16 = ml_dtypes.bfloat16


def _prep(x, edge_index, W1, b1, W2, b2, Wfc, bfc):
    """Host-side preprocessing: degrees, edge partitioning, layouts."""
    src = np.asarray(edge_index[0], dtype=np.int64)
    dst = np.asarray(edge_index[1], dtype=np.int64)

    deg = np.bincount(dst, minlength=N).astype(np.float64) + 1.0
    dinv = (1.0 / np.sqrt(deg)).astype(np.float32)

    x_s = (np.asarray(x, np.float32) * dinv[:, None]).astype(bf16)  # [N,128]

    core = dst // PC
    local = dst - core * PC
    blk = local // 128
    off = (local % 128).astype(np.float32)
    half = (src >= HALF).astype(np.int64)

    key = (core * NBLK + blk) * 2 + half
    order = np.argsort(key, kind="stable")
    cnt = np.bincount(key, minlength=N_CORES * NBLK * 2).reshape(
        N_CORES, NBLK, 2)
    # uniform chunk counts across cores (one SPMD program)
    CC = np.maximum(1, (cnt.max(axis=0) + CHUNK - 1) // CHUNK)  # [NBLK, 2]
    nch = (int(CC[:, 0].sum()), int(CC[:, 1].sum()))

    gstart = np.zeros(N_CORES * NBLK * 2 + 1, np.int64)
    np.cumsum(np.bincount(key, minlength=N_CORES * NBLK * 2), out=gstart[1:])
    src_sorted = src[order]
    off_sorted = off[order]

    in_maps = []
    for c in range(N_CORES):
        idx_streams = {}
        dst_streams = {}
        for h in (0, 1):
            nslots = nch[h] * CHUNK
            idx_arr = np.zeros(nslots, np.int16)
            off_arr = np.full(nslots, PAD_OFF, np.float32)
            pos = 0
            for b in range(NBLK):
                k = (c * NBLK + b) * 2 + h
                g0, g1 = gstart[k], gstart[k + 1]
                n = int(g1 - g0)
                idx_arr[pos:pos + n] = (src_sorted[g0:g1] - h * HALF).astype(
                    np.int16)
                off_arr[pos:pos + n] = off_sorted[g0:g1]
                pos += int(CC[b, h]) * CHUNK
            nb = (nch[h] + BPC - 1) // BPC
            idx_pad = np.zeros(nb * BATCH, np.int16)
            idx_pad[:nslots] = idx_arr
            # wrapped layout per 1024-batch: idx j -> partition j%16, col j//16
            w = idx_pad.reshape(nb, BATCH // 16, 16).transpose(0, 2, 1)
            idx_tile = np.tile(w, (1, 8, 1)).reshape(nb, 128, BATCH // 16)
            idx_tile = idx_tile.transpose(1, 0, 2).reshape(
                128, nb * BATCH // 16)
            idx_streams[h] = np.ascontiguousarray(idx_tile)
            # one-hot S per chunk: [128 edge-partitions, nch*128]
            S = (off_arr[:, None] == np.arange(128, dtype=np.float32)[None, :]
                 ).astype(bf16)
            S = S.reshape(nch[h], CHUNK, 128).transpose(1, 0, 2)
            dst_streams[h] = np.ascontiguousarray(
                S.reshape(CHUNK, nch[h] * 128))

        dl = dinv[c * PC:(c + 1) * PC]
        dinv_pad = np.zeros(PCP, np.float32)
        dinv_pad[:PC] = dl
        sq_pad = np.zeros(PCP, np.float32)
        sq_pad[:PC] = 1.0 / dl
        x_own = np.zeros((PCP, FEAT), bf16)
        x_own[:PC] = x_s[c * PC:(c + 1) * PC]

        im = {
            "x_s": x_s,
            "x_own": x_own,
            "idxA": idx_streams[0], "idxB": idx_streams[1],
            "sA": dst_streams[0], "sB": dst_streams[1],
            "W1": np.asarray(W1, np.float32).astype(bf16),
            "W2": np.asarray(W2, np.float32).astype(bf16),
            "Wfc": np.asarray(Wfc, np.float32).astype(bf16),
            "b1": np.asarray(b1, np.float32).astype(bf16)[None, :],
            "b2": np.asarray(b2, np.float32).astype(bf16)[None, :],
            "bfc": np.asarray(bfc, np.float32).astype(bf16)[None, :],
            "sqdeg": sq_pad.astype(bf16)[None, :],
            "dinv2T": np.ascontiguousarray(
                (dinv_pad ** 2).reshape(NBLK, 128).T.astype(np.float32)),
            "dinvT": np.ascontiguousarray(
                dinv_pad.reshape(NBLK, 128).T.astype(np.float32)),
            "ident": np.eye(128, dtype=bf16),
            "ones": np.ones((1, 128), bf16),
        }
        in_maps.append(im)

    meta = {"CC": CC, "nchA": nch[0], "nchB": nch[1]}
    return in_maps, meta


def _build(meta):
    import concourse.bacc as bacc
    import concourse.tile as tile
    from concourse import mybir

    CC = meta["CC"]
    nchA, nchB = meta["nchA"], meta["nchB"]
    nbA = (nchA + BPC - 1) // BPC
    nbB = (nchB + BPC - 1) // BPC

    nc = bacc.Bacc("TRN2", target_bir_lowering=False, debug=False,
                   num_devices=N_CORES, num_swdge_queues=4)
    f32, i16, bft = mybir.dt.float32, mybir.dt.int16, mybir.dt.bfloat16
    AO = mybir.AluOpType

    x_s = nc.dram_tensor("x_s", [N, FEAT], bft, kind="ExternalInput")
    x_own = nc.dram_tensor("x_own", [PCP, FEAT], bft, kind="ExternalInput")
    idxA = nc.dram_tensor("idxA", [128, nbA * BATCH // 16], i16,
                          kind="ExternalInput")
    idxB = nc.dram_tensor("idxB", [128, nbB * BATCH // 16], i16,
                          kind="ExternalInput")
    sA = nc.dram_tensor("sA", [128, nchA * 128], bft, kind="ExternalInput")
    sB = nc.dram_tensor("sB", [128, nchB * 128], bft, kind="ExternalInput")
    W1 = nc.dram_tensor("W1", [FEAT, HID], bft, kind="ExternalInput")
    W2 = nc.dram_tensor("W2", [HID, HID], bft, kind="ExternalInput")
    Wfc = nc.dram_tensor("Wfc", [HID, NCLS], bft, kind="ExternalInput")
    b1 = nc.dram_tensor("b1", [1, HID], bft, kind="ExternalInput")
    b2 = nc.dram_tensor("b2", [1, HID], bft, kind="ExternalInput")
    bfc = nc.dram_tensor("bfc", [1, NCLS], bft, kind="ExternalInput")
    sqdeg = nc.dram_tensor("sqdeg", [1, PCP], bft, kind="ExternalInput")
    dinv2T = nc.dram_tensor("dinv2T", [128, NBLK], f32, kind="ExternalInput")
    dinvT = nc.dram_tensor("dinvT", [128, NBLK], f32, kind="ExternalInput")
    ident = nc.dram_tensor("ident", [128, 128], bft, kind="ExternalInput")
    ones = nc.dram_tensor("ones", [1, 128], bft, kind="ExternalInput")

    out = nc.dram_tensor("out", [PCP, NCLS], f32, kind="ExternalOutput")

    y2_local = nc.dram_tensor("y2_local", [PC, 128], bft, kind="Internal")
    y2_full = nc.dram_tensor("y2_full", [N, 128], bft, kind="Internal",
                             addr_space="Shared")

    with tile.TileContext(nc) as tc:
        cp = tc.alloc_tile_pool(name="const", bufs=1)
        y2k = tc.alloc_tile_pool(name="y2keep", bufs=1)

        def load_const(name, dram, shape, dt):
            t = cp.tile(shape, dt, tag=name)
            nc.sync.dma_start(out=t[:], in_=dram[:, :])
            return t

        ident_t = load_const("ident", ident, [128, 128], bft)
        ones_t = load_const("ones", ones, [1, 128], bft)
        W1_t = load_const("W1", W1, [FEAT, HID], bft)
        W2_t = load_const("W2", W2, [HID, HID], bft)
        Wfc_t = load_const("Wfc", Wfc, [HID, NCLS], bft)
        b1_t = load_const("b1", b1, [1, HID], bft)
        b2_t = load_const("b2", b2, [1, HID], bft)
        bfc_t = load_const("bfc", bfc, [1, NCLS], bft)
        sq_t = load_const("sqdeg", sqdeg, [1, PCP], bft)
        d2_t = load_const("dinv2T", dinv2T, [128, NBLK], f32)
        d1_t = load_const("dinvT", dinvT, [128, NBLK], f32)
        idxA_t = load_const("idxA", idxA, [128, nbA * BATCH // 16], i16)
        idxB_t = load_const("idxB", idxB, [128, nbB * BATCH // 16], i16)

        gp = tc.alloc_tile_pool(name="g", bufs=8)
        sp = tc.alloc_tile_pool(name="s", bufs=4)
        xop = tc.alloc_tile_pool(name="xown", bufs=2)
        zxp = tc.alloc_tile_pool(name="zx", bufs=2, space="PSUM")
        z1p = tc.alloc_tile_pool(name="z1", bufs=2, space="PSUM")
        trp_ = tc.alloc_tile_pool(name="tr", bufs=1, space="PSUM")
        y2psp = tc.alloc_tile_pool(name="y2ps", bufs=1, space="PSUM")
        zxsbp = tc.alloc_tile_pool(name="zxsb", bufs=2)
        y2pp = tc.alloc_tile_pool(name="y2p", bufs=2)
        y2pTp = tc.alloc_tile_pool(name="y2pT", bufs=2)
        osbp = tc.alloc_tile_pool(name="osb", bufs=2)

        y2_tiles = []
        qctr = [0]

        def emit_layer(layer, gsrcA, gsrcB):
            batches = {0: {}, 1: {}}

            def get_batch(hlf, bi):
                d = batches[hlf]
                if bi in d:
                    return d[bi]
                g_t = gp.tile([128, BPC, FEAT], bft, tag="g")
                it = idxA_t if hlf == 0 else idxB_t
                srcap = gsrcA if hlf == 0 else gsrcB
                nc.gpsimd.dma_gather(
                    out_ap=g_t[:],
                    in_ap=srcap,
                    idxs_ap=it[:, bi * (BATCH // 16):(bi + 1) * (BATCH // 16)],
                    num_idxs=BATCH, num_idxs_reg=BATCH, elem_size=FEAT,
                    queue_num=qctr[0] % 4)
                qctr[0] += 1
                d[bi] = g_t
                for old in [k for k in d if k < bi - 2]:
                    del d[old]
                return g_t

            chunk_base = [0, 0]
            for b in range(NBLK):
                zx = zxp.tile([128, 128 if layer == 1 else HID], f32,
                              space="PSUM", tag="zx")
                first = True
                for hlf in (0, 1):
                    nch_blk = int(CC[b, hlf])
                    s_dram = sA if hlf == 0 else sB
                    c0 = chunk_base[hlf]
                    s_t = sp.tile([128, nch_blk * 128], bft, tag="s")
                    eng = nc.sync if (b + hlf) % 2 == 0 else nc.scalar
                    eng.dma_start(
                        out=s_t[:],
                        in_=s_dram[:, c0 * 128:(c0 + nch_blk) * 128])
                    for k in range(nch_blk):
                        ci = c0 + k
                        g_t = get_batch(hlf, ci // BPC)
                        cw = ci % BPC
                        if layer == 1:
                            # ZxT[feat, dst] += G^T @ S
                            nc.tensor.matmul(
                                out=zx[:], lhsT=g_t[:, cw, :],
                                rhs=s_t[:, k * 128:(k + 1) * 128],
                                start=first, stop=False)
                        else:
                            # Z2[dst, hid] += S^T @ G[:, :HID]
                            nc.tensor.matmul(
                                out=zx[:], lhsT=s_t[:, k * 128:(k + 1) * 128],
                                rhs=g_t[:, cw, 0:HID],
                                start=first, stop=False)
                        first = False
                    chunk_base[hlf] += nch_blk

                # self-loop term closes the accumulation group
                if layer == 1:
                    xo = xop.tile([128, FEAT], bft, tag="xown")
                    nc.sync.dma_start(
                        out=xo[:], in_=x_own[b * 128:(b + 1) * 128, :])
                    nc.tensor.matmul(out=zx[:], lhsT=xo[:], rhs=ident_t[:],
                                     start=first, stop=True)
                else:
                    nc.tensor.matmul(out=zx[:], lhsT=ident_t[:],
                                     rhs=y2_tiles[b][:, 0:HID],
                                     start=first, stop=True)

                if layer == 1:
                    zxs = zxsbp.tile([128, 128], bft, tag="zxsb")
                    if b % 2 == 0:
                        nc.vector.tensor_copy(out=zxs[:], in_=zx[:])
                    else:
                        nc.any.tensor_copy(out=zxs[:], in_=zx[:])
                    z1 = z1p.tile([128, HID], f32, space="PSUM", tag="z1")
                    nc.tensor.matmul(out=z1[:], lhsT=zxs[:], rhs=W1_t[:],
                                     start=True, stop=False)
                    nc.tensor.matmul(
                        out=z1[:], lhsT=sq_t[0:1, b * 128:(b + 1) * 128],
                        rhs=b1_t[:], start=False, stop=True)
                    y2p = y2pp.tile([128, HID], bft, tag="y2p")
                    nc.vector.tensor_scalar(
                        y2p[:], z1[:], 0.0, d2_t[:, b:b + 1], AO.max, AO.mult)
                    trp = trp_.tile([HID, 128], bft, space="PSUM", tag="tr")
                    nc.tensor.transpose(out=trp[:], in_=y2p[:],
                                        identity=ident_t[:])
                    y2pT = y2pTp.tile([HID, 128], bft, tag="y2pT")
                    nc.any.tensor_copy(out=y2pT[:], in_=trp[:])
                    y2ps = y2psp.tile([128, HID], f32, space="PSUM",
                                      tag="y2ps")
                    nc.tensor.matmul(out=y2ps[:], lhsT=y2pT[:], rhs=W2_t[:],
                                     start=True, stop=True)
                    y2s = y2k.tile([128, 128], bft, tag=f"y2_{b}")
                    nc.vector.tensor_copy(out=y2s[:, 0:HID], in_=y2ps[:])
                    y2_tiles.append(y2s)
                    r0 = b * 128
                    rows = min(128, PC - r0)
                    nc.sync.dma_start(out=y2_local[r0:r0 + rows, :],
                                      in_=y2s[0:rows, :])
                else:
                    h2 = y2pp.tile([128, HID], bft, tag="h2")
                    nc.vector.tensor_scalar(
                        h2[:], zx[:], 0.0, d1_t[:, b:b + 1], AO.max, AO.mult)
                    trp = trp_.tile([HID, 128], bft, space="PSUM", tag="tr")
                    nc.tensor.transpose(out=trp[:], in_=h2[:],
                                        identity=ident_t[:])
                    h2T = y2pTp.tile([HID, 128], bft, tag="h2T")
                    nc.any.tensor_copy(out=h2T[:], in_=trp[:])
                    op = z1p.tile([128, NCLS], f32, space="PSUM", tag="op")
                    nc.tensor.matmul(out=op[:], lhsT=h2T[:], rhs=Wfc_t[:],
                                     start=True, stop=False)
                    nc.tensor.matmul(out=op[:], lhsT=ones_t[:], rhs=bfc_t[:],
                                     start=False, stop=True)
                    osb = osbp.tile([128, NCLS], f32, tag="osb")
                    nc.vector.tensor_copy(out=osb[:], in_=op[:])
                    nc.sync.dma_start(out=out[b * 128:(b + 1) * 128, :],
                                      in_=osb[:])

        emit_layer(1, x_s[0:HALF, :], x_s[HALF:N, :])
        nc.gpsimd.collective_compute(
            "AllGather", AO.bypass,
            replica_groups=[list(range(N_CORES))],
            ins=[y2_local[:, :]], outs=[y2_full[:, :]])
        emit_layer(2, y2_full[0:HALF, :], y2_full[HALF:N, :])

        for p in (osbp, y2pTp, y2pp, zxsbp, y2psp, trp_, z1p, zxp, xop, sp,
                  gp, y2k, cp):
            p.release()

    nc.compile()
    return nc


def kernel(**inputs):
    from concourse import bass_utils

    in_maps, meta = _prep(**inputs)
    nc = _build(meta)
    res = bass_utils.run_bass_kernel_spmd(
        nc, in_maps, core_ids=list(range(N_CORES)))
    out = np.concatenate(
        [np.asarray(res.results[c]["out"])[:PC] for c in range(N_CORES)],
        axis=0)
    return out.astype(np.float32)


# revision 5
# speedup vs baseline: 1.3651x; 1.0273x over previous
"""GCN (2x GCNConv + FC) on Trainium2, 8-core SPMD Bass kernel.

Math (per layer): out = D^{-1/2} (A + I) D^{-1/2} (x @ W) + b, D = indeg + 1.
The two D^{-1/2} are folded into a host pre-scale of x rows and a device
post-scale of the aggregation (positive scales commute with relu).

Sharding: nodes split 8 ways by dst (6250/core). Per dst-block of 128 nodes,
edges are processed in 128-edge chunks: a one-hot selection matrix S (built
on the vector engine via is_equal against an iota row) scatters gathered
source rows into PSUM on the tensor engine. Source rows are fetched with
SWDGE dma_gather on 4 queues. Layer 1 aggregates raw pre-scaled x and applies
W1 after aggregation (linearity), so no y1 materialization is needed.
Hidden states for layer 2 are exchanged with an AllGather collective.
"""
import numpy as np
import ml_dtypes

N_CORES = 8
N = 50000
FEAT = 128
HID = 64
NCLS = 12
PC = N // N_CORES          # 6250 nodes per core
NBLK = (PC + 127) // 128   # 49 dst blocks per core
PCP = NBLK * 128           # 6272 padded rows
HALF = 25000               # int16 gather-index split point
CHUNK = 128
BATCH = 1024               # edges per dma_gather (HW cap at elem_size=128)
BPC = BATCH // CHUNK       # chunks per gather batch = 8
PAD_OFF = 200.0            # dst offset that matches no one-hot column
AGC = 4                    # AllGather chunks (pipelined)
SLICE = PCP // AGC         # rows per rank per AG slice = 1568
AGROWS = N_CORES * SLICE   # rows per AG slice output = 12544
NPOS = N_CORES * PCP       # position-space size = 50176
HALFP = NPOS // 2          # 25088, int16 split in position space

bf16 = ml_dtypes.bfloat16


def _prep(x, edge_index, W1, b1, W2, b2, Wfc, bfc):
    """Host-side preprocessing: degrees, edge partitioning, layouts."""
    src = np.asarray(edge_index[0], dtype=np.int64)
    dst = np.asarray(edge_index[1], dtype=np.int64)

    deg = np.bincount(dst, minlength=N).astype(np.float64) + 1.0
    dinv = (1.0 / np.sqrt(deg)).astype(np.float32)

    x_s = (np.asarray(x, np.float32) * dinv[:, None]).astype(bf16)  # [N,128]

    # position map: node (r, l) -> AG-chunked position
    rr = np.arange(N) // PC
    ll = np.arange(N) % PC
    posmap = (ll // SLICE) * AGROWS + rr * SLICE + (ll % SLICE)
    x_pos = np.zeros((NPOS, FEAT), bf16)
    x_pos[posmap] = x_s

    core = dst // PC
    local = dst - core * PC
    blk = local // 128
    off = (local % 128).astype(np.float32)
    pos = posmap[src]
    half = (pos >= HALFP).astype(np.int64)

    key = (core * NBLK + blk) * 2 + half
    order = np.argsort(key, kind="stable")
    cnt = np.bincount(key, minlength=N_CORES * NBLK * 2).reshape(
        N_CORES, NBLK, 2)
    # uniform chunk counts across cores (one SPMD program)
    CC = np.maximum(1, (cnt.max(axis=0) + CHUNK - 1) // CHUNK)  # [NBLK, 2]
    nch = (int(CC[:, 0].sum()), int(CC[:, 1].sum()))

    gstart = np.zeros(N_CORES * NBLK * 2 + 1, np.int64)
    np.cumsum(np.bincount(key, minlength=N_CORES * NBLK * 2), out=gstart[1:])
    pos_sorted = pos[order]
    off_sorted = off[order]

    in_maps = []
    for c in range(N_CORES):
        idx_streams = {}
        dst_streams = {}
        for h in (0, 1):
            nslots = nch[h] * CHUNK
            idx_arr = np.zeros(nslots, np.int16)
            off_arr = np.full(nslots, PAD_OFF, np.float32)
            p0 = 0
            for b in range(NBLK):
                k = (c * NBLK + b) * 2 + h
                g0, g1 = gstart[k], gstart[k + 1]
                n = int(g1 - g0)
                idx_arr[p0:p0 + n] = (pos_sorted[g0:g1] - h * HALFP).astype(
                    np.int16)
                off_arr[p0:p0 + n] = off_sorted[g0:g1]
                p0 += int(CC[b, h]) * CHUNK
            nb = (nch[h] + BPC - 1) // BPC
            idx_pad = np.zeros(nb * BATCH, np.int16)
            idx_pad[:nslots] = idx_arr
            # wrapped layout per 1024-batch: idx j -> partition j%16, col j//16
            w = idx_pad.reshape(nb, BATCH // 16, 16).transpose(0, 2, 1)
            idx_tile = np.tile(w, (1, 8, 1)).reshape(nb, 128, BATCH // 16)
            idx_tile = idx_tile.transpose(1, 0, 2).reshape(
                128, nb * BATCH // 16)
            idx_streams[h] = np.ascontiguousarray(idx_tile)
            # one-hot S per chunk: [128 edge-partitions, nch*128]
            S = (off_arr[:, None] == np.arange(128, dtype=np.float32)[None, :]
                 ).astype(bf16)
            S = S.reshape(nch[h], CHUNK, 128).transpose(1, 0, 2)
            dst_streams[h] = np.ascontiguousarray(
                S.reshape(CHUNK, nch[h] * 128))

        dl = dinv[c * PC:(c + 1) * PC]
        dinv_pad = np.zeros(PCP, np.float32)
        dinv_pad[:PC] = dl
        sq_pad = np.zeros(PCP, np.float32)
        sq_pad[:PC] = 1.0 / dl
        x_own = np.zeros((PCP, FEAT), bf16)
        x_own[:PC] = x_s[c * PC:(c + 1) * PC]

        im = {
            "x_s": x_pos,
            "x_own": x_own,
            "idxA": idx_streams[0], "idxB": idx_streams[1],
            "sA": dst_streams[0], "sB": dst_streams[1],
            "W1": np.asarray(W1, np.float32).astype(bf16),
            "W2": np.asarray(W2, np.float32).astype(bf16),
            "Wfc": np.asarray(Wfc, np.float32).astype(bf16),
            "b1": np.asarray(b1, np.float32).astype(bf16)[None, :],
            "b2": np.asarray(b2, np.float32).astype(bf16)[None, :],
            "bfc": np.asarray(bfc, np.float32).astype(bf16)[None, :],
            "sqdeg": sq_pad.astype(bf16)[None, :],
            "dinv2T": np.ascontiguousarray(
                (dinv_pad ** 2).reshape(NBLK, 128).T.astype(np.float32)),
            "dinvT": np.ascontiguousarray(
                dinv_pad.reshape(NBLK, 128).T.astype(np.float32)),
            "ident": np.eye(128, dtype=bf16),
            "ones": np.ones((1, 128), bf16),
        }
        in_maps.append(im)

    meta = {"CC": CC, "nchA": nch[0], "nchB": nch[1]}
    return in_maps, meta


def _build(meta):
    import concourse.bacc as bacc
    import concourse.tile as tile
    from concourse import mybir

    CC = meta["CC"]
    nchA, nchB = meta["nchA"], meta["nchB"]
    nbA = (nchA + BPC - 1) // BPC
    nbB = (nchB + BPC - 1) // BPC

    nc = bacc.Bacc("TRN2", target_bir_lowering=False, debug=False,
                   num_devices=N_CORES, num_swdge_queues=4,
                   dynamic_dma_scratch_size=65536)
    f32, i16, bft = mybir.dt.float32, mybir.dt.int16, mybir.dt.bfloat16
    AO = mybir.AluOpType

    x_s = nc.dram_tensor("x_s", [NPOS, FEAT], bft, kind="ExternalInput")
    x_own = nc.dram_tensor("x_own", [PCP, FEAT], bft, kind="ExternalInput")
    idxA = nc.dram_tensor("idxA", [128, nbA * BATCH // 16], i16,
                          kind="ExternalInput")
    idxB = nc.dram_tensor("idxB", [128, nbB * BATCH // 16], i16,
                          kind="ExternalInput")
    sA = nc.dram_tensor("sA", [128, nchA * 128], bft, kind="ExternalInput")
    sB = nc.dram_tensor("sB", [128, nchB * 128], bft, kind="ExternalInput")
    W1 = nc.dram_tensor("W1", [FEAT, HID], bft, kind="ExternalInput")
    W2 = nc.dram_tensor("W2", [HID, HID], bft, kind="ExternalInput")
    Wfc = nc.dram_tensor("Wfc", [HID, NCLS], bft, kind="ExternalInput")
    b1 = nc.dram_tensor("b1", [1, HID], bft, kind="ExternalInput")
    b2 = nc.dram_tensor("b2", [1, HID], bft, kind="ExternalInput")
    bfc = nc.dram_tensor("bfc", [1, NCLS], bft, kind="ExternalInput")
    sqdeg = nc.dram_tensor("sqdeg", [1, PCP], bft, kind="ExternalInput")
    dinv2T = nc.dram_tensor("dinv2T", [128, NBLK], f32, kind="ExternalInput")
    dinvT = nc.dram_tensor("dinvT", [128, NBLK], f32, kind="ExternalInput")
    ident = nc.dram_tensor("ident", [128, 128], bft, kind="ExternalInput")
    ones = nc.dram_tensor("ones", [1, 128], bft, kind="ExternalInput")

    out = nc.dram_tensor("out", [PCP, NCLS], f32, kind="ExternalOutput")

    y2_local = nc.dram_tensor("y2_local", [PCP, 128], bft, kind="Internal")
    y2_full = nc.dram_tensor("y2_full", [NPOS, 128], bft, kind="Internal",
                             addr_space="Shared")

    with tile.TileContext(nc) as tc:
        cp = tc.alloc_tile_pool(name="const", bufs=1)
        y2k = tc.alloc_tile_pool(name="y2keep", bufs=1)

        def load_const(name, dram, shape, dt):
            t = cp.tile(shape, dt, tag=name)
            nc.sync.dma_start(out=t[:], in_=dram[:, :])
            return t

        ident_t = load_const("ident", ident, [128, 128], bft)
        ones_t = load_const("ones", ones, [1, 128], bft)
        W1_t = load_const("W1", W1, [FEAT, HID], bft)
        W2_t = load_const("W2", W2, [HID, HID], bft)
        Wfc_t = load_const("Wfc", Wfc, [HID, NCLS], bft)
        b1_t = load_const("b1", b1, [1, HID], bft)
        b2_t = load_const("b2", b2, [1, HID], bft)
        bfc_t = load_const("bfc", bfc, [1, NCLS], bft)
        sq_t = load_const("sqdeg", sqdeg, [1, PCP], bft)
        d2_t = load_const("dinv2T", dinv2T, [128, NBLK], f32)
        d1_t = load_const("dinvT", dinvT, [128, NBLK], f32)
        idxA_t = load_const("idxA", idxA, [128, nbA * BATCH // 16], i16)
        idxB_t = load_const("idxB", idxB, [128, nbB * BATCH // 16], i16)

        gp = tc.alloc_tile_pool(name="g", bufs=8)
        sp = tc.alloc_tile_pool(name="s", bufs=4)
        xop = tc.alloc_tile_pool(name="xown", bufs=2)
        zxp = tc.alloc_tile_pool(name="zx", bufs=2, space="PSUM")
        z1p = tc.alloc_tile_pool(name="z1", bufs=2, space="PSUM")
        trp_ = tc.alloc_tile_pool(name="tr", bufs=1, space="PSUM")
        y2psp = tc.alloc_tile_pool(name="y2ps", bufs=1, space="PSUM")
        zxsbp = tc.alloc_tile_pool(name="zxsb", bufs=2)
        y2pp = tc.alloc_tile_pool(name="y2p", bufs=2)
        y2pTp = tc.alloc_tile_pool(name="y2pT", bufs=2)
        osbp = tc.alloc_tile_pool(name="osb", bufs=2)

        y2_tiles = []
        qctr = [0]

        def emit_layer(layer, gsrcA, gsrcB):
            batches = {0: {}, 1: {}}

            def get_batch(hlf, bi):
                d = batches[hlf]
                if bi in d:
                    return d[bi]
                g_t = gp.tile([128, BPC, FEAT], bft, tag="g")
                it = idxA_t if hlf == 0 else idxB_t
                srcap = gsrcA if hlf == 0 else gsrcB
                nc.gpsimd.dma_gather(
                    out_ap=g_t[:],
                    in_ap=srcap,
                    idxs_ap=it[:, bi * (BATCH // 16):(bi + 1) * (BATCH // 16)],
                    num_idxs=BATCH, num_idxs_reg=BATCH, elem_size=FEAT,
                    queue_num=qctr[0] % 4)
                qctr[0] += 1
                d[bi] = g_t
                for old in [k for k in d if k < bi - 2]:
                    del d[old]
                return g_t

            chunk_base = [0, 0]
            for b in range(NBLK):
                zx = zxp.tile([128, 128 if layer == 1 else HID], f32,
                              space="PSUM", tag="zx")
                first = True
                for hlf in (0, 1):
                    nch_blk = int(CC[b, hlf])
                    s_dram = sA if hlf == 0 else sB
                    c0 = chunk_base[hlf]
                    s_t = sp.tile([128, nch_blk * 128], bft, tag="s")
                    eng = nc.sync if (b + hlf) % 2 == 0 else nc.scalar
                    eng.dma_start(
                        out=s_t[:],
                        in_=s_dram[:, c0 * 128:(c0 + nch_blk) * 128])
                    for k in range(nch_blk):
                        ci = c0 + k
                        g_t = get_batch(hlf, ci // BPC)
                        cw = ci % BPC
                        if layer == 1:
                            # ZxT[feat, dst] += G^T @ S
                            nc.tensor.matmul(
                                out=zx[:], lhsT=g_t[:, cw, :],
                                rhs=s_t[:, k * 128:(k + 1) * 128],
                                start=first, stop=False)
                        else:
                            # Z2[dst, hid] += S^T @ G[:, :HID]
                            nc.tensor.matmul(
                                out=zx[:], lhsT=s_t[:, k * 128:(k + 1) * 128],
                                rhs=g_t[:, cw, 0:HID],
                                start=first, stop=False)
                        first = False
                    chunk_base[hlf] += nch_blk

                # self-loop term closes the accumulation group
                if layer == 1:
                    xo = xop.tile([128, FEAT], bft, tag="xown")
                    nc.sync.dma_start(
                        out=xo[:], in_=x_own[b * 128:(b + 1) * 128, :])
                    nc.tensor.matmul(out=zx[:], lhsT=xo[:], rhs=ident_t[:],
                                     start=first, stop=True)
                else:
                    nc.tensor.matmul(out=zx[:], lhsT=ident_t[:],
                                     rhs=y2_tiles[b][:, 0:HID],
                                     start=first, stop=True)

                if layer == 1:
                    zxs = zxsbp.tile([128, 128], bft, tag="zxsb")
                    if b % 2 == 0:
                        nc.vector.tensor_copy(out=zxs[:], in_=zx[:])
                    else:
                        nc.any.tensor_copy(out=zxs[:], in_=zx[:])
                    z1 = z1p.tile([128, HID], f32, space="PSUM", tag="z1")
                    nc.tensor.matmul(out=z1[:], lhsT=zxs[:], rhs=W1_t[:],
                                     start=True, stop=False)
                    nc.tensor.matmul(
                        out=z1[:], lhsT=sq_t[0:1, b * 128:(b + 1) * 128],
                        rhs=b1_t[:], start=False, stop=True)
                    y2p = y2pp.tile([128, HID], bft, tag="y2p")
                    nc.vector.tensor_scalar(
                        y2p[:], z1[:], 0.0, d2_t[:, b:b + 1], AO.max, AO.mult)
                    trp = trp_.tile([HID, 128], bft, space="PSUM", tag="tr")
                    nc.tensor.transpose(out=trp[:], in_=y2p[:],
                                        identity=ident_t[:])
                    y2pT = y2pTp.tile([HID, 128], bft, tag="y2pT")
                    nc.any.tensor_copy(out=y2pT[:], in_=trp[:])
                    y2ps = y2psp.tile([128, HID], f32, space="PSUM",
                                      tag="y2ps")
                    nc.tensor.matmul(out=y2ps[:], lhsT=y2pT[:], rhs=W2_t[:],
                                     start=True, stop=True)
                    y2s = y2k.tile([128, 128], bft, tag=f"y2_{b}")
                    nc.vector.tensor_copy(out=y2s[:, 0:HID], in_=y2ps[:])
                    y2_tiles.append(y2s)
                    r0 = b * 128
                    nc.sync.dma_start(out=y2_local[r0:r0 + 128, :],
                                      in_=y2s[:, :])
                else:
                    h2 = y2pp.tile([128, HID], bft, tag="h2")
                    nc.vector.tensor_scalar(
                        h2[:], zx[:], 0.0, d1_t[:, b:b + 1], AO.max, AO.mult)
                    trp = trp_.tile([HID, 128], bft, space="PSUM", tag="tr")
                    nc.tensor.transpose(out=trp[:], in_=h2[:],
                                        identity=ident_t[:])
                    h2T = y2pTp.tile([HID, 128], bft, tag="h2T")
                    nc.any.tensor_copy(out=h2T[:], in_=trp[:])
                    op = z1p.tile([128, NCLS], f32, space="PSUM", tag="op")
                    nc.tensor.matmul(out=op[:], lhsT=h2T[:], rhs=Wfc_t[:],
                                     start=True, stop=False)
                    nc.tensor.matmul(out=op[:], lhsT=ones_t[:], rhs=bfc_t[:],
                                     start=False, stop=True)
                    osb = osbp.tile([128, NCLS], f32, tag="osb")
                    nc.vector.tensor_copy(out=osb[:], in_=op[:])
                    nc.sync.dma_start(out=out[b * 128:(b + 1) * 128, :],
                                      in_=osb[:])

        emit_layer(1, x_s[0:HALFP, :], x_s[HALFP:NPOS, :])
        for k in range(AGC):
            nc.gpsimd.collective_compute(
                "AllGather", AO.bypass,
                replica_groups=[list(range(N_CORES))],
                ins=[y2_local[k * SLICE:(k + 1) * SLICE, :]],
                outs=[y2_full[k * AGROWS:(k + 1) * AGROWS, :]])
        emit_layer(2, y2_full[0:HALFP, :], y2_full[HALFP:NPOS, :])

        for p in (osbp, y2pTp, y2pp, zxsbp, y2psp, trp_, z1p, zxp, xop, sp,
                  gp, y2k, cp):
            p.release()

    nc.compile()
    return nc


def kernel(**inputs):
    from concourse import bass_utils

    in_maps, meta = _prep(**inputs)
    nc = _build(meta)
    res = bass_utils.run_bass_kernel_spmd(
        nc, in_maps, core_ids=list(range(N_CORES)))
    out = np.concatenate(
        [np.asarray(res.results[c]["out"])[:PC] for c in range(N_CORES)],
        axis=0)
    return out.astype(np.float32)


# revision 6
# speedup vs baseline: 1.5234x; 1.1160x over previous
"""GCN (2x GCNConv + FC) on Trainium2, 8-core SPMD Bass kernel.

Math (per layer): out = D^{-1/2} (A + I) D^{-1/2} (x @ W) + b, D = indeg + 1.
The two D^{-1/2} are folded into a host pre-scale of x rows and a device
post-scale of the aggregation (positive scales commute with relu).

Sharding: nodes split 8 ways by dst (6250/core). Per dst-block of 128 nodes,
edges are processed in 128-edge chunks: a one-hot selection matrix S (built
on the vector engine via is_equal against an iota row) scatters gathered
source rows into PSUM on the tensor engine. Source rows are fetched with
SWDGE dma_gather on 4 queues. Layer 1 aggregates raw pre-scaled x and applies
W1 after aggregation (linearity), so no y1 materialization is needed.
Hidden states for layer 2 are exchanged with an AllGather collective.
"""
import numpy as np
import ml_dtypes

N_CORES = 8
N = 50000
FEAT = 128
HID = 64
NCLS = 12
PC = N // N_CORES          # 6250 nodes per core
NBLK = (PC + 127) // 128   # 49 dst blocks per core
PCP = NBLK * 128           # 6272 padded rows
HALF = 25000               # int16 gather-index split point
CHUNK = 128
BATCH = 1024               # edges per dma_gather (HW cap at elem_size=128)
BPC = BATCH // CHUNK       # chunks per gather batch = 8
PAD_OFF = 200.0            # dst offset that matches no one-hot column
AGC = 2                    # AllGather chunks (pipelined)
SLICE = PCP // AGC         # rows per rank per AG slice = 1568
AGROWS = N_CORES * SLICE   # rows per AG slice output = 12544
NPOS = N_CORES * PCP       # position-space size = 50176
HALFP = NPOS // 2          # 25088, int16 split in position space

bf16 = ml_dtypes.bfloat16


def _prep(x, edge_index, W1, b1, W2, b2, Wfc, bfc):
    """Host-side preprocessing: degrees, edge partitioning, layouts."""
    src = np.asarray(edge_index[0], dtype=np.int64)
    dst = np.asarray(edge_index[1], dtype=np.int64)

    deg = np.bincount(dst, minlength=N).astype(np.float64) + 1.0
    dinv = (1.0 / np.sqrt(deg)).astype(np.float32)

    x_s = (np.asarray(x, np.float32) * dinv[:, None]).astype(bf16)  # [N,128]

    # position map: node (r, l) -> AG-chunked position
    rr = np.arange(N) // PC
    ll = np.arange(N) % PC
    posmap = (ll // SLICE) * AGROWS + rr * SLICE + (ll % SLICE)
    x_pos = np.zeros((NPOS, FEAT), bf16)
    x_pos[posmap] = x_s

    core = dst // PC
    local = dst - core * PC
    blk = local // 128
    off = (local % 128).astype(np.float32)
    pos = posmap[src]
    half = (pos >= HALFP).astype(np.int64)

    key = (core * NBLK + blk) * 2 + half
    order = np.argsort(key, kind="stable")
    cnt = np.bincount(key, minlength=N_CORES * NBLK * 2).reshape(
        N_CORES, NBLK, 2)
    # uniform chunk counts across cores (one SPMD program)
    CC = np.maximum(1, (cnt.max(axis=0) + CHUNK - 1) // CHUNK)  # [NBLK, 2]
    nch = (int(CC[:, 0].sum()), int(CC[:, 1].sum()))

    gstart = np.zeros(N_CORES * NBLK * 2 + 1, np.int64)
    np.cumsum(np.bincount(key, minlength=N_CORES * NBLK * 2), out=gstart[1:])
    pos_sorted = pos[order]
    off_sorted = off[order]

    in_maps = []
    for c in range(N_CORES):
        idx_streams = {}
        dst_streams = {}
        for h in (0, 1):
            nslots = nch[h] * CHUNK
            idx_arr = np.zeros(nslots, np.int16)
            off_arr = np.full(nslots, PAD_OFF, np.float32)
            p0 = 0
            for b in range(NBLK):
                k = (c * NBLK + b) * 2 + h
                g0, g1 = gstart[k], gstart[k + 1]
                n = int(g1 - g0)
                idx_arr[p0:p0 + n] = (pos_sorted[g0:g1] - h * HALFP).astype(
                    np.int16)
                off_arr[p0:p0 + n] = off_sorted[g0:g1]
                p0 += int(CC[b, h]) * CHUNK
            nb = (nch[h] + BPC - 1) // BPC
            idx_pad = np.zeros(nb * BATCH, np.int16)
            idx_pad[:nslots] = idx_arr
            # wrapped layout per 1024-batch: idx j -> partition j%16, col j//16
            w = idx_pad.reshape(nb, BATCH // 16, 16).transpose(0, 2, 1)
            idx_tile = np.tile(w, (1, 8, 1)).reshape(nb, 128, BATCH // 16)
            idx_tile = idx_tile.transpose(1, 0, 2).reshape(
                128, nb * BATCH // 16)
            idx_streams[h] = np.ascontiguousarray(idx_tile)
            # one-hot S per chunk: [128 edge-partitions, nch*128]
            S = (off_arr[:, None] == np.arange(128, dtype=np.float32)[None, :]
                 ).astype(bf16)
            S = S.reshape(nch[h], CHUNK, 128).transpose(1, 0, 2)
            dst_streams[h] = np.ascontiguousarray(
                S.reshape(CHUNK, nch[h] * 128))

        dl = dinv[c * PC:(c + 1) * PC]
        dinv_pad = np.zeros(PCP, np.float32)
        dinv_pad[:PC] = dl
        sq_pad = np.zeros(PCP, np.float32)
        sq_pad[:PC] = 1.0 / dl
        x_own = np.zeros((PCP, FEAT), bf16)
        x_own[:PC] = x_s[c * PC:(c + 1) * PC]

        im = {
            "x_s": x_pos,
            "x_own": x_own,
            "idxA": idx_streams[0], "idxB": idx_streams[1],
            "sA": dst_streams[0], "sB": dst_streams[1],
            "W1": np.asarray(W1, np.float32).astype(bf16),
            "W2": np.asarray(W2, np.float32).astype(bf16),
            "Wfc": np.asarray(Wfc, np.float32).astype(bf16),
            "b1": np.asarray(b1, np.float32).astype(bf16)[None, :],
            "b2": np.asarray(b2, np.float32).astype(bf16)[None, :],
            "bfc": np.asarray(bfc, np.float32).astype(bf16)[None, :],
            "sqdeg": sq_pad.astype(bf16)[None, :],
            "dinv2T": np.ascontiguousarray(
                (dinv_pad ** 2).reshape(NBLK, 128).T.astype(np.float32)),
            "dinvT": np.ascontiguousarray(
                dinv_pad.reshape(NBLK, 128).T.astype(np.float32)),
            "ident": np.eye(128, dtype=bf16),
            "ones": np.ones((1, 128), bf16),
        }
        in_maps.append(im)

    meta = {"CC": CC, "nchA": nch[0], "nchB": nch[1]}
    return in_maps, meta


def _build(meta):
    import concourse.bacc as bacc
    import concourse.tile as tile
    from concourse import mybir

    CC = meta["CC"]
    nchA, nchB = meta["nchA"], meta["nchB"]
    nbA = (nchA + BPC - 1) // BPC
    nbB = (nchB + BPC - 1) // BPC

    nc = bacc.Bacc("TRN2", target_bir_lowering=False, debug=False,
                   num_devices=N_CORES, num_swdge_queues=4,
                   dynamic_dma_scratch_size=65536)
    f32, i16, bft = mybir.dt.float32, mybir.dt.int16, mybir.dt.bfloat16
    AO = mybir.AluOpType

    x_s = nc.dram_tensor("x_s", [NPOS, FEAT], bft, kind="ExternalInput")
    x_own = nc.dram_tensor("x_own", [PCP, FEAT], bft, kind="ExternalInput")
    idxA = nc.dram_tensor("idxA", [128, nbA * BATCH // 16], i16,
                          kind="ExternalInput")
    idxB = nc.dram_tensor("idxB", [128, nbB * BATCH // 16], i16,
                          kind="ExternalInput")
    sA = nc.dram_tensor("sA", [128, nchA * 128], bft, kind="ExternalInput")
    sB = nc.dram_tensor("sB", [128, nchB * 128], bft, kind="ExternalInput")
    W1 = nc.dram_tensor("W1", [FEAT, HID], bft, kind="ExternalInput")
    W2 = nc.dram_tensor("W2", [HID, HID], bft, kind="ExternalInput")
    Wfc = nc.dram_tensor("Wfc", [HID, NCLS], bft, kind="ExternalInput")
    b1 = nc.dram_tensor("b1", [1, HID], bft, kind="ExternalInput")
    b2 = nc.dram_tensor("b2", [1, HID], bft, kind="ExternalInput")
    bfc = nc.dram_tensor("bfc", [1, NCLS], bft, kind="ExternalInput")
    sqdeg = nc.dram_tensor("sqdeg", [1, PCP], bft, kind="ExternalInput")
    dinv2T = nc.dram_tensor("dinv2T", [128, NBLK], f32, kind="ExternalInput")
    dinvT = nc.dram_tensor("dinvT", [128, NBLK], f32, kind="ExternalInput")
    ident = nc.dram_tensor("ident", [128, 128], bft, kind="ExternalInput")
    ones = nc.dram_tensor("ones", [1, 128], bft, kind="ExternalInput")

    out = nc.dram_tensor("out", [PCP, NCLS], f32, kind="ExternalOutput")

    y2_local = nc.dram_tensor("y2_local", [PCP, 128], bft, kind="Internal")
    y2_full = nc.dram_tensor("y2_full", [NPOS, 128], bft, kind="Internal",
                             addr_space="Shared")

    with tile.TileContext(nc) as tc:
        cp = tc.alloc_tile_pool(name="const", bufs=1)
        y2k = tc.alloc_tile_pool(name="y2keep", bufs=1)

        def load_const(name, dram, shape, dt):
            t = cp.tile(shape, dt, tag=name)
            nc.sync.dma_start(out=t[:], in_=dram[:, :])
            return t

        ident_t = load_const("ident", ident, [128, 128], bft)
        ones_t = load_const("ones", ones, [1, 128], bft)
        W1_t = load_const("W1", W1, [FEAT, HID], bft)
        W2_t = load_const("W2", W2, [HID, HID], bft)
        Wfc_t = load_const("Wfc", Wfc, [HID, NCLS], bft)
        b1_t = load_const("b1", b1, [1, HID], bft)
        b2_t = load_const("b2", b2, [1, HID], bft)
        bfc_t = load_const("bfc", bfc, [1, NCLS], bft)
        sq_t = load_const("sqdeg", sqdeg, [1, PCP], bft)
        d2_t = load_const("dinv2T", dinv2T, [128, NBLK], f32)
        d1_t = load_const("dinvT", dinvT, [128, NBLK], f32)
        idxA_t = load_const("idxA", idxA, [128, nbA * BATCH // 16], i16)
        idxB_t = load_const("idxB", idxB, [128, nbB * BATCH // 16], i16)

        gp = tc.alloc_tile_pool(name="g", bufs=8)
        sp = tc.alloc_tile_pool(name="s", bufs=4)
        xop = tc.alloc_tile_pool(name="xown", bufs=2)
        zxp = tc.alloc_tile_pool(name="zx", bufs=2, space="PSUM")
        z1p = tc.alloc_tile_pool(name="z1", bufs=2, space="PSUM")
        trp_ = tc.alloc_tile_pool(name="tr", bufs=1, space="PSUM")
        y2psp = tc.alloc_tile_pool(name="y2ps", bufs=1, space="PSUM")
        zxsbp = tc.alloc_tile_pool(name="zxsb", bufs=2)
        y2pp = tc.alloc_tile_pool(name="y2p", bufs=2)
        y2pTp = tc.alloc_tile_pool(name="y2pT", bufs=2)
        osbp = tc.alloc_tile_pool(name="osb", bufs=2)

        y2_tiles = []
        qctr = [0]

        def emit_layer(layer, gsrcA, gsrcB):
            batches = {0: {}, 1: {}}

            def get_batch(hlf, bi):
                d = batches[hlf]
                if bi in d:
                    return d[bi]
                g_t = gp.tile([128, BPC, FEAT], bft, tag="g")
                it = idxA_t if hlf == 0 else idxB_t
                srcap = gsrcA if hlf == 0 else gsrcB
                nc.gpsimd.dma_gather(
                    out_ap=g_t[:],
                    in_ap=srcap,
                    idxs_ap=it[:, bi * (BATCH // 16):(bi + 1) * (BATCH // 16)],
                    num_idxs=BATCH, num_idxs_reg=BATCH, elem_size=FEAT,
                    queue_num=qctr[0] % 4)
                qctr[0] += 1
                d[bi] = g_t
                for old in [k for k in d if k < bi - 2]:
                    del d[old]
                return g_t

            chunk_base = [0, 0]
            for b in range(NBLK):
                zx = zxp.tile([128, 128 if layer == 1 else HID], f32,
                              space="PSUM", tag="zx")
                first = True
                for hlf in (0, 1):
                    nch_blk = int(CC[b, hlf])
                    s_dram = sA if hlf == 0 else sB
                    c0 = chunk_base[hlf]
                    s_t = sp.tile([128, nch_blk * 128], bft, tag="s")
                    eng = nc.sync if (b + hlf) % 2 == 0 else nc.scalar
                    eng.dma_start(
                        out=s_t[:],
                        in_=s_dram[:, c0 * 128:(c0 + nch_blk) * 128])
                    for k in range(nch_blk):
                        ci = c0 + k
                        g_t = get_batch(hlf, ci // BPC)
                        cw = ci % BPC
                        if layer == 1:
                            # ZxT[feat, dst] += G^T @ S
                            nc.tensor.matmul(
                                out=zx[:], lhsT=g_t[:, cw, :],
                                rhs=s_t[:, k * 128:(k + 1) * 128],
                                start=first, stop=False)
                        else:
                            # Z2[dst, hid] += S^T @ G[:, :HID]
                            nc.tensor.matmul(
                                out=zx[:], lhsT=s_t[:, k * 128:(k + 1) * 128],
                                rhs=g_t[:, cw, 0:HID],
                                start=first, stop=False)
                        first = False
                    chunk_base[hlf] += nch_blk

                # self-loop term closes the accumulation group
                if layer == 1:
                    xo = xop.tile([128, FEAT], bft, tag="xown")
                    nc.sync.dma_start(
                        out=xo[:], in_=x_own[b * 128:(b + 1) * 128, :])
                    nc.tensor.matmul(out=zx[:], lhsT=xo[:], rhs=ident_t[:],
                                     start=first, stop=True)
                else:
                    nc.tensor.matmul(out=zx[:], lhsT=ident_t[:],
                                     rhs=y2_tiles[b][:, 0:HID],
                                     start=first, stop=True)

                if layer == 1:
                    zxs = zxsbp.tile([128, 128], bft, tag="zxsb")
                    if b % 2 == 0:
                        nc.vector.tensor_copy(out=zxs[:], in_=zx[:])
                    else:
                        nc.any.tensor_copy(out=zxs[:], in_=zx[:])
                    z1 = z1p.tile([128, HID], f32, space="PSUM", tag="z1")
                    nc.tensor.matmul(out=z1[:], lhsT=zxs[:], rhs=W1_t[:],
                                     start=True, stop=False)
                    nc.tensor.matmul(
                        out=z1[:], lhsT=sq_t[0:1, b * 128:(b + 1) * 128],
                        rhs=b1_t[:], start=False, stop=True)
                    y2p = y2pp.tile([128, HID], bft, tag="y2p")
                    nc.vector.tensor_scalar(
                        y2p[:], z1[:], 0.0, d2_t[:, b:b + 1], AO.max, AO.mult)
                    trp = trp_.tile([HID, 128], bft, space="PSUM", tag="tr")
                    nc.tensor.transpose(out=trp[:], in_=y2p[:],
                                        identity=ident_t[:])
                    y2pT = y2pTp.tile([HID, 128], bft, tag="y2pT")
                    nc.any.tensor_copy(out=y2pT[:], in_=trp[:])
                    y2ps = y2psp.tile([128, HID], f32, space="PSUM",
                                      tag="y2ps")
                    nc.tensor.matmul(out=y2ps[:], lhsT=y2pT[:], rhs=W2_t[:],
                                     start=True, stop=True)
                    y2s = y2k.tile([128, 128], bft, tag=f"y2_{b}")
                    nc.vector.tensor_copy(out=y2s[:, 0:HID], in_=y2ps[:])
                    y2_tiles.append(y2s)
                    r0 = b * 128
                    nc.sync.dma_start(out=y2_local[r0:r0 + 128, :],
                                      in_=y2s[:, :])
                else:
                    h2 = y2pp.tile([128, HID], bft, tag="h2")
                    nc.vector.tensor_scalar(
                        h2[:], zx[:], 0.0, d1_t[:, b:b + 1], AO.max, AO.mult)
                    trp = trp_.tile([HID, 128], bft, space="PSUM", tag="tr")
                    nc.tensor.transpose(out=trp[:], in_=h2[:],
                                        identity=ident_t[:])
                    h2T = y2pTp.tile([HID, 128], bft, tag="h2T")
                    nc.any.tensor_copy(out=h2T[:], in_=trp[:])
                    op = z1p.tile([128, NCLS], f32, space="PSUM", tag="op")
                    nc.tensor.matmul(out=op[:], lhsT=h2T[:], rhs=Wfc_t[:],
                                     start=True, stop=False)
                    nc.tensor.matmul(out=op[:], lhsT=ones_t[:], rhs=bfc_t[:],
                                     start=False, stop=True)
                    osb = osbp.tile([128, NCLS], f32, tag="osb")
                    nc.vector.tensor_copy(out=osb[:], in_=op[:])
                    nc.sync.dma_start(out=out[b * 128:(b + 1) * 128, :],
                                      in_=osb[:])

        emit_layer(1, x_s[0:HALFP, :], x_s[HALFP:NPOS, :])
        for k in range(AGC):
            nc.gpsimd.collective_compute(
                "AllGather", AO.bypass,
                replica_groups=[list(range(N_CORES))],
                ins=[y2_local[k * SLICE:(k + 1) * SLICE, :]],
                outs=[y2_full[k * AGROWS:(k + 1) * AGROWS, :]])
        emit_layer(2, y2_full[0:HALFP, :], y2_full[HALFP:NPOS, :])

        for p in (osbp, y2pTp, y2pp, zxsbp, y2psp, trp_, z1p, zxp, xop, sp,
                  gp, y2k, cp):
            p.release()

    nc.compile()
    return nc


def kernel(**inputs):
    from concourse import bass_utils

    in_maps, meta = _prep(**inputs)
    nc = _build(meta)
    res = bass_utils.run_bass_kernel_spmd(
        nc, in_maps, core_ids=list(range(N_CORES)))
    out = np.concatenate(
        [np.asarray(res.results[c]["out"])[:PC] for c in range(N_CORES)],
        axis=0)
    return out.astype(np.float32)
